# revision 51
# baseline (speedup 1.0000x reference)
"""Trainium2 Bass kernel for nn_Attention_45835890982922.

Dense multi-head attention block:
    qkv = x @ w_qkv ; q,k layernormed per head (eps=1e-5), q scaled by D^-0.5
    out = softmax(q k^T) v ; y = concat_heads(out) @ w_proj + b_proj

Sharding over 8 NeuronCores: hybrid batch x tensor-parallel.
Core c handles batch b = c//2 and heads [6*(c%2), 6*(c%2)+6).
Each core computes a partial y^T (its 6 heads through the matching
w_proj rows); the host sums the two partials per batch and adds b_proj.

On-chip layout is feature-major (transposed): x^T, q^T, k^T [D, tokens],
so every matmul contraction lives on the partition axis with no per-head
transposes.  Softmax runs without max-subtraction (|S| <= ~8 after LN),
with the normalization sum obtained from an extra all-ones column
appended to v; the division is folded into the PSUM->SBUF epilogue of
the attention-output matmul.

dtypes: float32r (TensorE reduced fp32, ~1.5e-4) for qkv/S/stats/proj
matmuls, bf16 for exp(S) probabilities and v, fp32 accumulation in PSUM.
"""

from contextlib import ExitStack

import numpy as np

import concourse.bacc as bacc
import concourse.tile as tile
import concourse.mybir as mybir
from concourse.bass_utils import run_bass_kernel_spmd
import ml_dtypes

NP_BF16 = ml_dtypes.bfloat16

F32 = mybir.dt.float32
F32R = mybir.dt.float32r
BF16 = mybir.dt.bfloat16
I16 = mybir.dt.int16
OP = mybir.AluOpType
AF = mybir.ActivationFunctionType

# Schraudolph exp on DVE: bf16 bits of e^x ~= round(2^7/ln2 * x + B_SCH),
# computed as fp32->int16 then bit-reinterpreted as bf16. B_SCH tuned on the
# full-model error metric (softmax cancels the systematic part).
A_SCH = 2.0 ** 7 / 0.6931471805599453
B_SCH = 16249.5
# exp engine split, interleaved so ACT and DVE drain S tiles concurrently
DVE_KT = frozenset((2, 5, 8, 11, 14))

B, N, C, H, D = 4, 2048, 768, 12, 64
HL = H // 2              # 6 heads per core
CL = HL * D              # 384 local feature rows
P = 128
NKT = N // P             # 16 key tiles
QC = 1024                # query chunk for attention
NQC = N // QC
CT = C // P              # 6 contraction tiles over C
FT_QK = 2 * CL // P      # 6 feature tiles for q|k
KT3 = CL // P            # 3 contraction tiles over CL
LN_EPS = 1e-5
SCALE = float(D) ** -0.5

USE_BF16_IN = False      # ship x/wqk/wv (and ident) as bf16
COPIES_ACT = True        # alternate PSUM->SBUF copies between DVE and ACT
EPI_COPY = True          # copy ps_o to SBUF before the divide (frees PSUM early)
EPI_POOL = False         # (no win: slower Pool TT delays ocs release)
PD_BUFS = 3              # epilogue scratch depth (ocopy/recip/recipb)
PE_BUFS = 4              # projection y staging depth
PSE_BUFS = 6             # projection PSUM accumulator depth
PC_BUFS = 1              # apply scratch depth (tdiff/ftmp)
MERGE_MR = False         # (no win: ldweights are already shared; kept for reference)

# ablation knobs (timing experiments only; wrong output when not default)
ABL_HEADS = HL
ABL_QKV = True
ABL_ATTN = True
ABL_PROJ = True
ABL_SHIFT = False
ABL_EXPFN = True
ABL_LN = True
PSS_BUFS = 2
PSO_BUFS = 2
HPARTS = 2
ABL_EPI = True


def _build(trivial_beta: bool, repeat: int = 1, trivial_gamma: bool = True):
    nc = bacc.Bacc("TRN2", target_bir_lowering=False, debug=False, num_devices=8)

    XDT = BF16 if USE_BF16_IN else F32
    WDT = BF16 if USE_BF16_IN else F32R
    x_d = nc.dram_tensor("x", [N, C], XDT, kind="ExternalInput").ap()
    wqk_d = nc.dram_tensor("wqk", [C, 2 * CL], WDT, kind="ExternalInput").ap()
    wv_d = nc.dram_tensor("wv", [C, CL], WDT, kind="ExternalInput").ap()
    wp_d = nc.dram_tensor("wp", [CL, C], F32R, kind="ExternalInput").ap()
    ident_d = nc.dram_tensor("ident", [P, P], XDT, kind="ExternalInput").ap()
    bd6_d = nc.dram_tensor("bd6", [CL, HL], F32R, kind="ExternalInput").ap()
    bc6_d = nc.dram_tensor("bc6", [HL, CL], F32R, kind="ExternalInput").ap()
    gb_d = nc.dram_tensor("gb", [CL, 4], F32, kind="ExternalInput").ap()
    sh_d = nc.dram_tensor("sh64", [P, P], F32R, kind="ExternalInput").ap()
    y_d = nc.dram_tensor("y", [C, N], F32, kind="ExternalOutput").ap()

    with tile.TileContext(nc) as tc, ExitStack() as top:
        top.enter_context(
            nc.allow_low_precision(reason="f32r/bf16 staging is intentional")
        )
        const = top.enter_context(tc.tile_pool(name="const", bufs=1))

        ident = const.tile([P, P], XDT)
        nc.sync.dma_start(ident[:], ident_d)
        bd6 = const.tile([P, KT3, HL], F32R)
        nc.sync.dma_start(bd6[:], bd6_d.rearrange("(t p) h -> p t h", p=P))
        bc6 = const.tile([HL, CL], F32R)
        nc.sync.dma_start(bc6[:], bc6_d)
        gb = const.tile([P, KT3, 4], F32)
        nc.sync.dma_start(gb[:], gb_d.rearrange("(t p) c -> p t c", p=P))
        sh64 = const.tile([P, P], F32R)
        nc.sync.dma_start(sh64[:], sh_d)

        nc._xdt, nc._wdt = XDT, WDT
        for rep in range(repeat):
            _emit_iteration(
                nc, tc, rep, trivial_beta, trivial_gamma,
                x_d, wqk_d, wv_d, wp_d, y_d, ident, bd6, bc6, gb, sh64,
            )

    nc.compile()
    return nc


def _emit_iteration(nc, tc, rep, trivial_beta, trivial_gamma,
                    x_d, wqk_d, wv_d, wp_d, y_d, ident, bd6, bc6, gb, sh64):
    trivial_ln = trivial_beta and trivial_gamma
    XDT, WDT = nc._xdt, nc._wdt
    XT_DT = BF16 if XDT == BF16 else F32R
    HK = NKT // HPARTS
    with ExitStack() as top:
        vp = top.enter_context(tc.tile_pool(name=f"vpool{rep}", bufs=1))
        # v token-major bf16 with per-head all-ones column: [p, ttile, h*65+e]
        v_sb = vp.tile([P, NKT, HL * 65], BF16)
        v_view = v_sb[:].rearrange("p t (h e) -> p t h e", h=HL)
        nc.gpsimd.memset(v_view[:, :, :, 64:65], 1.0)

        # q^T | k^T feature-major accumulator: [p, ft, tokens]; ft 0-2 q, 3-5 k.
        # LayerNorm is applied in-place, so this same tile later holds qhat/khat.
        qkp = top.enter_context(tc.tile_pool(name=f"qkraw{rep}", bufs=1))
        qk_fts = [
            qkp.tile([P, N], F32R, name=f"qk_ft{ft}_{rep}") for ft in range(FT_QK)
        ]

        class _FtView:
            """hat[p_slice, ft, col_slice] -> per-ft tile AP."""
            def __init__(self, tiles):
                self.tiles = tiles
            def __getitem__(self, idx):
                p, ft, col = idx
                return self.tiles[ft][p, col]

        qk_raw = _FtView(qk_fts)
        hat = qk_raw
        # LN smalls live from phase A stats through the last inline apply
        sAC = top.enter_context(ExitStack())
        smp = sAC.enter_context(tc.tile_pool(name=f"smalls{rep}", bufs=1))
        if MERGE_MR:
            # mu and rstd interleaved per token so a [6, 512, 2] slice
            # flattens to a contiguous [6, 1024] matmul moving operand
            sm_mr = [smp.tile([HL, N, 2], F32R, tag=f"mr{s}",
                              name=f"sm_mr{s}_{rep}") for s in range(2)]
            sm_mu = [t[:, :, 0] for t in sm_mr]
            sm_rst = [t[:, :, 1] for t in sm_mr]
        else:
            sm_mu = [smp.tile([HL, N], F32R, tag=f"mu{s}",
                              name=f"sm_mu{s}_{rep}")[:, :] for s in range(2)]
            sm_rst = [smp.tile([HL, N], F32R, tag=f"rst{s}",
                               name=f"sm_rst{s}_{rep}")[:, :] for s in range(2)]
        # broadcast PSUM for LN applies: 2 banks, alive into phase D (created
        # before psA/psB so pool stack order matches release order)
        psC = sAC.enter_context(tc.tile_pool(name=f"psC{rep}", bufs=1, space="PSUM"))
        pC = sAC.enter_context(tc.tile_pool(name=f"phC{rep}", bufs=PC_BUFS))
        # ============ phase A: x^T, qkv, and inline LN stats ============
        with ExitStack() as sA:
            pA = sA.enter_context(tc.tile_pool(name=f"phA{rep}", bufs=1))
            pAx = sA.enter_context(tc.tile_pool(name=f"phAx{rep}", bufs=2))
            psA = sA.enter_context(tc.tile_pool(name=f"psA{rep}", bufs=2, space="PSUM"))

            wqk_r = pA.tile([P, CT, 2 * CL], WDT)
            wv_r = pA.tile([P, CT, CL], WDT)

            x_t = pA.tile([P, CT, N], XT_DT)          # [c%128, ctile, token]
            x_rows = x_d.rearrange("(t p) c -> p t c", p=P)
            ncp = 0
            with ExitStack() as sB:
                psB = sB.enter_context(
                    tc.tile_pool(name=f"psB{rep}", bufs=2, space="PSUM"))
                pB = sB.enter_context(tc.tile_pool(name=f"phB{rep}", bufs=2))
                for tb in range(N // 512):
                    xs = pAx.tile([P, 4, C], XDT, tag="xslice")
                    nc.sync.dma_start(xs[:], x_rows[:, tb * 4:(tb + 1) * 4, :])
                    if tb == 0:
                        nc.sync.dma_start(
                            wqk_r[:], wqk_d.rearrange("(t p) f -> p t f", p=P))
                    elif tb == 1:
                        nc.sync.dma_start(
                            wv_r[:], wv_d.rearrange("(t p) f -> p t f", p=P))
                    bsl = slice(tb * 512, (tb + 1) * 512)
                    # bf16 transposes pack 2 ct per PSUM bank; f32 needs
                    # a bank per ct to keep psA at 4 banks
                    TG = 2 if XDT == BF16 else 1
                    for cg in range(CT // TG):
                        pst = psA.tile([P, TG * 512], XDT, tag="ps_tr")
                        for c2 in range(TG):
                            for j in range(4):
                                nc.tensor.transpose(
                                    pst[:, c2 * 512 + j * P:
                                        c2 * 512 + (j + 1) * P],
                                    xs[:, j, (TG * cg + c2) * P:
                                       (TG * cg + c2 + 1) * P], ident[:],
                                )
                        eng = (nc.vector if (ncp % 2 == 0 or not COPIES_ACT)
                               else nc.scalar)
                        ncp += 1
                        cp = (eng.tensor_copy if eng is nc.vector else eng.copy)
                        cp(x_t[:, TG * cg:TG * cg + TG, bsl],
                           pst[:].rearrange("p (c n) -> p c n", c=TG))

                    for ft in range(FT_QK if ABL_QKV else 0):
                        ps = psA.tile([P, 512], F32, tag="ps_qkv")
                        for kt in range(CT):
                            nc.tensor.matmul(
                                ps[:],
                                wqk_r[:, kt, ft * P:(ft + 1) * P],
                                x_t[:, kt, bsl],
                                start=(kt == 0),
                                stop=(kt == CT - 1),
                            )
                        if ncp % 2 == 0 or not COPIES_ACT:
                            nc.vector.tensor_copy(qk_raw[:, ft, bsl], ps[:])
                        else:
                            nc.scalar.copy(qk_raw[:, ft, bsl], ps[:])
                        ncp += 1

                    # LN stats for this token block.  s=0 stats fold 1/SCALE
                    # (mu) and 1/SCALE^2 (E[x^2]) so the finalize pass needs
                    # no extra multiply: rstd_q comes out pre-scaled by SCALE.
                    for s in range(2 if ABL_LN else 0):
                        mu_c = 1.0 / D / (SCALE if s == 0 and trivial_ln else 1.0)
                        e2_c = 1.0 / D / (SCALE ** 2 if s == 0 and trivial_ln else 1.0)
                        psm = psB.tile([HL, 512], F32, tag="ps_stat")
                        for kt in range(KT3):
                            nc.tensor.matmul(
                                psm[:], bd6[:, kt, :],
                                qk_raw[:, 3 * s + kt, bsl],
                                start=(kt == 0), stop=(kt == KT3 - 1),
                            )
                        nc.vector.tensor_scalar_mul(
                            sm_mu[s][:, bsl], psm[:], mu_c)
                        psq = psB.tile([HL, 512], F32, tag="ps_stat")
                        for kt in range(KT3):
                            sq = pB.tile([P, 512], F32R, tag="sq")
                            nc.scalar.square(
                                sq[:], qk_raw[:, 3 * s + kt, bsl])
                            nc.tensor.matmul(
                                psq[:], bd6[:, kt, :], sq[:],
                                start=(kt == 0), stop=(kt == KT3 - 1),
                            )
                        nc.vector.tensor_scalar_mul(
                            sm_rst[s][:, bsl], psq[:], e2_c)

            # ======== prelude: stat finalize + v + pair-0 LN apply ========
            # (v matmuls keep the PE busy while DVE/Pool finish the stats
            # and apply LN to ft 0/3, which the first attention head needs)
            def finalize(s, half):
                sl = slice(half * 1024, (half + 1) * 1024)
                eps = LN_EPS / (SCALE ** 2 if s == 0 and trivial_ln else 1.0)
                tmp = pC.tile([HL, 1024], F32, tag="ftmp")
                nc.vector.tensor_tensor(
                    tmp[:], sm_mu[s][:, sl], sm_mu[s][:, sl], OP.mult)
                nc.vector.scalar_tensor_tensor(
                    tmp[:], sm_rst[s][:, sl], eps, tmp[:],
                    op0=OP.add, op1=OP.subtract)
                nc.scalar.activation(tmp[:], tmp[:], AF.Sqrt)
                nc.vector.reciprocal(sm_rst[s][:, sl], tmp[:])
                if s == 0 and not trivial_ln:
                    nc.vector.tensor_scalar_mul(
                        sm_rst[0][:, sl], sm_rst[0][:, sl], SCALE)

            def apply_chunk(ft, chnk, eng):
                """LN-apply one 512-token chunk of one ft (trivial path)."""
                s = ft // 3
                blk = ft % 3
                sl = slice(chnk * 512, (chnk + 1) * 512)
                if MERGE_MR:
                    # mu|rst interleaved smalls: ONE broadcast matmul (and
                    # one ldweights) yields both per-token vectors
                    bc = psC.tile([P, 512, 2], F32, tag="bc",
                                  name=f"bc_{rep}_{ft}_{chnk}")
                    nc.tensor.matmul(
                        bc[:], bc6[:, blk * P:(blk + 1) * P],
                        sm_mr[s][:, sl, :], start=True, stop=True)
                    if s == 0:
                        eng.tensor_tensor(
                            hat[:, ft, sl], qk_raw[:, ft, sl], bc[:, :, 1],
                            OP.mult)
                        return
                    tdiff = pC.tile([P, 512], F32, tag="tdiff")
                    eng.tensor_tensor(
                        tdiff[:], qk_raw[:, ft, sl], bc[:, :, 0], OP.subtract)
                    eng.tensor_tensor(
                        hat[:, ft, sl], tdiff[:], bc[:, :, 1], OP.mult)
                    return
                brs = psC.tile([P, 512], F32, tag="bcr")
                nc.tensor.matmul(
                    brs[:], bc6[:, blk * P:(blk + 1) * P],
                    sm_rst[s][:, sl], start=True, stop=True)
                if s == 0:
                    # khat is exactly zero-sum over D, so S is unchanged by
                    # q's mean: no mu_q subtraction needed.
                    eng.tensor_tensor(
                        hat[:, ft, sl], qk_raw[:, ft, sl], brs[:], OP.mult)
                    return
                bmu = psC.tile([P, 512], F32, tag="bcb")
                nc.tensor.matmul(
                    bmu[:], bc6[:, blk * P:(blk + 1) * P],
                    sm_mu[s][:, sl], start=True, stop=True)
                tdiff = pC.tile([P, 512], F32, tag="tdiff")
                eng.tensor_tensor(
                    tdiff[:], qk_raw[:, ft, sl], bmu[:], OP.subtract)
                eng.tensor_tensor(hat[:, ft, sl], tdiff[:], brs[:], OP.mult)

            if trivial_ln and ABL_LN:
                items = []
                items.append(lambda: finalize(1, 0))
                items.append(lambda: finalize(1, 1))
                for chnk in range(4):
                    items.append(
                        lambda c=chnk: apply_chunk(3, c, nc.vector))
                items.append(lambda: finalize(0, 0))
                items.append(lambda: finalize(0, 1))
                for chnk in range(4):
                    items.append(
                        lambda c=chnk: apply_chunk(0, c, nc.vector))
                items.reverse()
            else:
                items = []
                if ABL_LN:
                    for s in range(2):
                        finalize(s, 0)
                        finalize(s, 1)

            sV = ExitStack()
            psV = sV.enter_context(
                tc.tile_pool(name=f"psV{rep}", bufs=2, space="PSUM"))
            for tt in range(NKT):
                if items:
                    items.pop()()
                psv = psV.tile([P, CL], F32, tag="ps_v")
                for kt in range(CT):
                    nc.tensor.matmul(
                        psv[:],
                        x_t[:, kt, tt * P:(tt + 1) * P],
                        wv_r[:, kt, :],
                        start=(kt == 0),
                        stop=(kt == CT - 1),
                    )
                vcp = nc.scalar.copy if COPIES_ACT else nc.vector.tensor_copy
                vcp(
                    v_view[:, tt, :, 0:64],
                    psv[:].rearrange("p (h d) -> p h d", h=HL),
                )
            while items:
                items.pop()()
            sV.close()

        # ======== non-trivial LN fallback: block apply (general gamma/beta) ====
        if not trivial_ln and ABL_LN:
            with ExitStack() as sC:
                for ft in [0, 3, 1, 4, 2, 5][:FT_QK]:
                    s = ft // 3
                    blk = ft % 3
                    for nh in range(N // 512):
                        sl = slice(nh * 512, (nh + 1) * 512)
                        brs = psC.tile([P, 512], F32, tag="bcr")
                        nc.tensor.matmul(
                            brs[:], bc6[:, blk * P:(blk + 1) * P],
                            sm_rst[s][:, sl], start=True, stop=True)
                        bmu = psC.tile([P, 512], F32, tag="bcb")
                        nc.tensor.matmul(
                            bmu[:], bc6[:, blk * P:(blk + 1) * P],
                            sm_mu[s][:, sl], start=True, stop=True)
                        tdiff = pC.tile([P, 512], F32, tag="tdiff")
                        nc.vector.tensor_tensor(
                            tdiff[:], qk_raw[:, ft, sl], bmu[:], OP.subtract)
                        nc.vector.scalar_tensor_tensor(
                            hat[:, ft, sl], tdiff[:],
                            gb[:, blk, 2 * s:2 * s + 1], brs[:],
                            op0=OP.mult, op1=OP.mult)
                        if not trivial_beta:
                            nc.vector.tensor_scalar_add(
                                hat[:, ft, sl], hat[:, ft, sl],
                                gb[:, blk, 2 * s + 1:2 * s + 2])

        # ================ phase D: attention ================
        outp = top.enter_context(tc.tile_pool(name=f"outT{rep}", bufs=1))
        out_fts = [
            outp.tile([P, N], F32R, name=f"out_ft{t}_{rep}") for t in range(KT3)
        ]
        out_t = _FtView(out_fts)                      # out^T feature-major
        # prefetch the projection weights while attention runs
        wpp = top.enter_context(tc.tile_pool(name=f"wpp{rep}", bufs=1))
        wp_r = wpp.tile([P, KT3, C], F32R)
        nc.sync.dma_start(wp_r[:], wp_d.rearrange("(t p) f -> p t f", p=P))

        # inline LN-apply schedule: pair p's chunks run during pair p-1's
        # four attention windows (2 items per window), k ft first
        apply_sched = {}
        if trivial_ln and ABL_LN:
            for pair in (1, 2):
                slots = [(2 * pair - 2, 0), (2 * pair - 2, 1),
                         (2 * pair - 1, 0), (2 * pair - 1, 1)]
                items = []
                for chnk in range(4):
                    items.append((3 + pair, chnk, nc.vector))
                for chnk in range(4):
                    items.append((pair, chnk, nc.vector))
                for i, slot in enumerate(slots):
                    apply_sched[slot] = items[2 * i:2 * i + 2]

        with ExitStack() as sD:
            expp = sD.enter_context(tc.tile_pool(name=f"expp{rep}", bufs=2 * HPARTS - 1))
            pD = sD.enter_context(tc.tile_pool(name=f"phD{rep}", bufs=PD_BUFS))
            psS = sD.enter_context(tc.tile_pool(name=f"psS{rep}", bufs=PSS_BUFS, space="PSUM"))
            psO = sD.enter_context(tc.tile_pool(name=f"psO{rep}", bufs=PSO_BUFS, space="PSUM"))

            def emit_chunk(cur, prev, applies=()):
                """Interleave cur's S+exp with prev's PV at kt granularity so
                the PE falls through to PV whenever S stalls on exp drain."""
                exp_halves = []
                if cur is not None:
                    h, qc = cur
                    ht = h // 2
                    hr = 64 * (h % 2)
                if prev is not None:
                    ph, pqc, peh = prev
                    ps_os = [
                        psO.tile([65, 512], F32, tag="ps_o",
                                 name=f"ps_o_{rep}_{ph}_{pqc}_{i}")
                        for i in range(QC // 512)
                    ]
                for kt in range(NKT):
                    if kt == 2 and len(applies) > 0:
                        apply_chunk(*applies[0])
                    if kt == 9 and len(applies) > 1:
                        apply_chunk(*applies[1])
                    if cur is not None:
                        if kt % HK == 0:
                            exp_h = expp.tile(
                                [P, HK, QC], BF16, tag="exp",
                                name=f"exp_{rep}_{h}_{qc}_{kt // HK}",
                            )
                            exp_halves.append(exp_h)
                        lhs = hat[hr:hr + 64, 3 + ht, kt * P:(kt + 1) * P]
                        ps_st = psS.tile([P, QC], F32, tag="ps_s")
                        for nk in range(QC // 512):
                            nc.tensor.matmul(
                                ps_st[:, nk * 512:(nk + 1) * 512],
                                lhs,
                                hat[hr:hr + 64, ht,
                                    qc * QC + nk * 512:qc * QC + (nk + 1) * 512],
                                start=True,
                                stop=True,
                            )
                        dst = exp_h[:, kt % HK, :]
                        if kt not in DVE_KT:
                            nc.scalar.activation(
                                dst, ps_st[:],
                                AF.Exp if ABL_EXPFN else AF.Copy,
                            )
                        else:
                            # Schraudolph exp on DVE: int16 bits that
                            # reinterpret as bf16 e^x.
                            nc.vector.tensor_scalar(
                                dst.bitcast(I16), ps_st[:], A_SCH, B_SCH,
                                op0=OP.mult, op1=OP.add,
                            )
                    if prev is not None:
                        for nk in range(QC // 512):
                            nc.tensor.matmul(
                                ps_os[nk][:],
                                v_view[:, kt, ph, :],
                                peh[kt // HK][:, kt % HK,
                                              nk * 512:(nk + 1) * 512],
                                start=(kt == 0),
                                stop=(kt == NKT - 1),
                            )
                if prev is not None:
                    pht = ph // 2
                    phr = 64 * (ph % 2)
                    # copy PSUM->SBUF first so the ps_o banks free quickly
                    # (the next chunk's PV needs them at its window start);
                    # the divide then runs from SBUF off the critical path
                    if EPI_COPY:
                        ocs = []
                        for nk in range(QC // 512):
                            oc = pD.tile([65, 512], F32, tag="ocopy",
                                         name=f"oc_{rep}_{ph}_{pqc}_{nk}")
                            nc.vector.tensor_copy(oc[:], ps_os[nk][:])
                            ocs.append(oc)
                    else:
                        ocs = ps_os
                    for nk in range(QC // 512):
                        osl = slice(pqc * QC + nk * 512,
                                    pqc * QC + (nk + 1) * 512)
                        if ABL_EPI:
                            rc = pD.tile([1, 512], F32, tag="recip")
                            nc.vector.reciprocal(rc[:], ocs[nk][64:65, :])
                            rcb = pD.tile([64, 512], F32, tag="recipb")
                            nc.gpsimd.partition_broadcast(rcb[:], rc[:])
                            # all-SBUF after the pre-copy, so the spare Pool
                            # engine can absorb it and free DVE for exps
                            te = nc.gpsimd if (EPI_POOL and EPI_COPY) else nc.vector
                            te.tensor_tensor(
                                out_t[phr:phr + 64, pht, osl],
                                ocs[nk][0:64, :],
                                rcb[:],
                                OP.mult,
                            )
                        else:
                            nc.vector.tensor_copy(
                                out_t[phr:phr + 64, pht, osl],
                                ocs[nk][0:64, :],
                            )
                return exp_halves

            pending = None
            for h in range(ABL_HEADS if ABL_ATTN else 0):
                for qc in range(NQC):
                    eh = emit_chunk((h, qc), pending,
                                    apply_sched.get((h, qc), ()))
                    pending = (h, qc, eh)
            if pending is not None:
                emit_chunk(None, pending)

        # ================ phase E: output projection ================
        with ExitStack() as sE:
            pE = sE.enter_context(tc.tile_pool(name=f"phE{rep}", bufs=PE_BUFS))
            psE = sE.enter_context(tc.tile_pool(name=f"psE{rep}", bufs=PSE_BUFS, space="PSUM"))
            for mt in range(C // P if ABL_PROJ else 0):
                y_sb = pE.tile([P, N], F32, tag="y")
                for nk in range(N // 512):
                    ps_y = psE.tile([P, 512], F32, tag="ps_y")
                    for kt in range(KT3):
                        nc.tensor.matmul(
                            ps_y[:],
                            wp_r[:, kt, mt * P:(mt + 1) * P],
                            out_t[:, kt, nk * 512:(nk + 1) * 512],
                            start=(kt == 0),
                            stop=(kt == KT3 - 1),
                        )
                    if nk % 2 == 0 or not COPIES_ACT:
                        nc.vector.tensor_copy(
                            y_sb[:, nk * 512:(nk + 1) * 512], ps_y[:]
                        )
                    else:
                        nc.scalar.copy(y_sb[:, nk * 512:(nk + 1) * 512], ps_y[:])
                nc.sync.dma_start(y_d[mt * P:(mt + 1) * P, :], y_sb[:])


def _host_prep(x, w_qkv, q_gamma, q_beta, k_gamma, k_beta, w_proj):
    """Per-core input maps."""
    idt = NP_BF16 if USE_BF16_IN else np.float32
    wdt = NP_BF16 if USE_BF16_IN else np.float32
    ident = np.eye(P, dtype=np.float32).astype(idt)
    sh64 = np.zeros((P, P), dtype=np.float32)
    sh64[(np.arange(P) + 64) % P, np.arange(P)] = 1.0
    bd6 = np.zeros((CL, HL), dtype=np.float32)
    for h in range(HL):
        bd6[h * D:(h + 1) * D, h] = 1.0
    bc6 = np.ascontiguousarray(bd6.T)
    in_maps = []
    for c in range(8):
        b = c // 2
        half = c % 2
        heads = range(HL * half, HL * half + HL)
        wq = np.concatenate([w_qkv[:, h * D:(h + 1) * D] for h in heads], axis=1)
        wk = np.concatenate(
            [w_qkv[:, C + h * D:C + (h + 1) * D] for h in heads], axis=1
        )
        wv = np.concatenate(
            [w_qkv[:, 2 * C + h * D:2 * C + (h + 1) * D] for h in heads], axis=1
        ).astype(wdt)
        wqk = np.ascontiguousarray(
            np.concatenate([wq, wk], axis=1)).astype(wdt)
        wp = np.ascontiguousarray(w_proj[CL * half:CL * half + CL, :])
        gb = np.stack(
            [
                np.tile(q_gamma, HL),
                np.tile(q_beta, HL) * SCALE,
                np.tile(k_gamma, HL),
                np.tile(k_beta, HL),
            ],
            axis=1,
        ).astype(np.float32)
        in_maps.append(
            {
                "x": np.ascontiguousarray(x[b]).astype(idt),
                "wqk": wqk,
                "wv": np.ascontiguousarray(wv),
                "wp": wp,
                "ident": ident,
                "sh64": sh64,
                "bd6": bd6,
                "bc6": bc6,
                "gb": gb,
            }
        )
    return in_maps


def kernel(x, w_qkv, q_gamma, q_beta, k_gamma, k_beta, w_proj, b_proj):
    x = np.asarray(x, dtype=np.float32)
    w_qkv = np.asarray(w_qkv, dtype=np.float32)
    q_gamma = np.asarray(q_gamma, dtype=np.float32)
    q_beta = np.asarray(q_beta, dtype=np.float32)
    k_gamma = np.asarray(k_gamma, dtype=np.float32)
    k_beta = np.asarray(k_beta, dtype=np.float32)
    w_proj = np.asarray(w_proj, dtype=np.float32)
    b_proj = np.asarray(b_proj, dtype=np.float32)

    trivial_beta = bool(np.all(q_beta == 0.0) and np.all(k_beta == 0.0))
    trivial_gamma = bool(np.all(q_gamma == 1.0) and np.all(k_gamma == 1.0))
    nc = _build(trivial_beta, trivial_gamma=trivial_gamma)
    in_maps = _host_prep(x, w_qkv, q_gamma, q_beta, k_gamma, k_beta, w_proj)
    res = run_bass_kernel_spmd(nc, in_maps, core_ids=list(range(8)))

    y = np.empty((B, N, C), dtype=np.float32)
    for b in range(B):
        yt = res.results[2 * b]["y"] + res.results[2 * b + 1]["y"]
        y[b] = yt.T + b_proj[None, :]
    return y


if __name__ == "__main__":
    rng = np.random.default_rng(0)
    out = kernel(
        rng.standard_normal((B, N, C), dtype=np.float32),
        (rng.standard_normal((C, 3 * C)) * C ** -0.5).astype(np.float32),
        np.ones(D, np.float32),
        np.zeros(D, np.float32),
        np.ones(D, np.float32),
        np.zeros(D, np.float32),
        (rng.standard_normal((C, C)) * C ** -0.5).astype(np.float32),
        np.zeros(C, np.float32),
    )
    print("ok", out.shape, float(np.abs(out).mean()))

